# revision 4
# baseline (speedup 1.0000x reference)
"""Trainium2 Bass kernel v2 for nn_DataReuploadingTorso.

Circuit = 17 diagonal phase layers D_k interleaved with H^{x8}. Four of the
D_k (the theta1-only layers) are batch-independent: they are folded into
dense complex matrices G_l = H D H on the host, applied as f16 PE matmuls
with PSUM-accumulated real/imag parts (no elementwise work). The remaining
13 x-dependent layers use: q = W^T x (fp32r PE), magic-number rne range
reduction (ACT copy-bias + DVE STT), S = sin(2*pi*q), C = cos via
Sin(pi/2 - 2*pi*|g|), then an f16 complex modulation split across
ACT/DVE/Pool.

Sharding: pure data parallel over batch, 8 cores x 2048.
State layout: [128 partitions = s mod 128, (s_hi half | batch)] packed
[128, 2F], f16.
"""
import numpy as np

import concourse.bass as bass
import concourse.mybir as mybir
import concourse.tile as tile
from concourse.bass_utils import run_bass_kernel_spmd

N_CORES = 8
B_TOTAL = 16384
B_CORE = B_TOTAL // N_CORES      # 2048
F = 512                          # batch per chunk
NCH = B_CORE // F                # 4 chunks
DIM = 256
N_Q = 8

F32 = mybir.dt.float32
F32R = mybir.dt.float32r
F16 = mybir.dt.float16
AOT = mybir.AluOpType
ACTF = mybir.ActivationFunctionType

PI = float(np.pi)
MAGIC = float(np.float32(1.5 * 2 ** 23))

XDEP = [0, 1, 3, 4, 5, 7, 8, 9, 11, 12, 13, 15, 16]   # x-dependent layers
CONST = {2: 0, 6: 1, 10: 2, 14: 3}                     # layer idx -> G idx


# ----------------------------------------------------------------- host tables
def _build_host_tables(theta, omega):
    theta = np.asarray(theta, np.float64)              # (8, 5, 3)
    omega = np.asarray(omega, np.float64).reshape(5, 8, 3)

    idx = np.arange(DIM)
    beta = np.stack([(idx >> (7 - q)) & 1 for q in range(N_Q)], 0)   # (8, 256)
    sgn = (2 * beta - 1).astype(np.float64)

    def czterm(pairs):
        t = np.zeros(DIM)
        for a, b in pairs:
            t += np.pi * (beta[a] * beta[b])
        return t
    cz_even = czterm([(0, 1), (2, 3), (4, 5), (6, 7)])
    cz_odd = czterm([(1, 2), (3, 4), (5, 6)])

    steps = []
    for l in range(4):
        A = np.zeros((12, 8)); k = np.zeros(8)
        for q in range(8):
            A[3 * (q % 4) + 1, q] = omega[l, q, 1]
        steps.append((A, k, None))
        A = np.zeros((12, 8)); k = np.zeros(8)
        for q in range(8):
            A[3 * (q % 4) + 2, q] = omega[l, q, 2]
            k[q] = theta[q, l, 0]
        steps.append((A, k, None))
        A = np.zeros((12, 8)); k = theta[:, l, 1].copy()
        steps.append((A, k, None))
        A = np.zeros((12, 8)); k = theta[:, l, 2].copy()
        for q in range(8):
            A[3 * (q % 4) + 0, q] = omega[l + 1, q, 0]
        if l + 1 == 4:
            k += theta[:, 4, 0]
        steps.append((A, k, cz_even if l % 2 == 0 else cz_odd))
    A = np.zeros((12, 8)); k = theta[:, 4, 1].copy()
    for q in range(8):
        A[3 * (q % 4) + 1, q] = omega[4, q, 1]
    steps.append((A, k, None))

    inv2pi = 1.0 / (2.0 * np.pi)
    W = np.zeros((len(XDEP), 13, DIM))
    for i, si in enumerate(XDEP):
        A, k, cz = steps[si]
        W[i, :12] = (A @ (sgn * 0.5)) * inv2pi
        c = k @ (sgn * 0.5)
        if cz is not None:
            c = c + cz
        W[i, 12] = (np.mod(c + np.pi, 2 * np.pi) - np.pi) * inv2pi

    # H^{x8} sign matrix and unitary
    sp = np.arange(DIM)
    pop = np.zeros((DIM, DIM), np.int64)
    for q in range(8):
        pop += np.outer((sp >> q) & 1, (sp >> q) & 1)
    H8 = np.where(pop % 2 == 0, 1.0, -1.0)
    Hu = H8 / 16.0                                     # exact unitary
    M = (H8[:128, :128] / 16.0).astype(np.float32)     # H^{x7}sgn / 16

    # merged G_l = Hu diag(e^{i Phi_c}) Hu for the theta1-only layers
    G_blocks = []                                      # (4, hp, h) -> (GrT, GiT)
    for si in sorted(CONST):
        _, k, _ = steps[si]
        phic = k @ (sgn * 0.5)                         # (256,)
        G = (Hu * np.exp(1j * phic)[None, :]) @ Hu     # Hu @ diag @ Hu
        blocks = []
        for hp in range(2):
            for h in range(2):
                B = G[hp * 128:(hp + 1) * 128, h * 128:(h + 1) * 128]
                blocks.append((B.real.T.copy(), B.imag.T.copy()))
        G_blocks.append(blocks)

    Z = np.stack([1.0 - 2.0 * ((idx >> (7 - q)) & 1) for q in range(8)], 1)
    return W.astype(np.float32), M, G_blocks, Z.astype(np.float32)


# -------------------------------------------------------------- device program
def _legalize_waits(nc, limit=1):
    """walrus codegen allows only one embedded sync-wait on several TRN2
    instruction encodings. Hoist excess waits onto same-engine NoOps."""
    def fix_block(blk):
        new_insts = []
        for ins in blk.instructions:
            si = getattr(ins, "sync_info", None)
            waits = list(si.on_wait) if si and si.on_wait else []
            if len(waits) > limit:
                keep = waits[-limit:]
                for j, w in enumerate(waits[:-limit]):
                    new_insts.append(mybir.InstNoOp(
                        name=f"{ins.name}-w{j}",
                        engine=ins.engine,
                        sync_info=mybir.SyncInfo(on_wait=[w], on_update=[]),
                    ))
                si.on_wait = keep
            new_insts.append(ins)
        blk.instructions = new_insts
        for sb in getattr(blk, "blocks", None) or []:
            fix_block(sb)
    for f in nc.m.functions:
        for blk in f.blocks:
            fix_block(blk)


def _build_program():
    nc = bass.Bass("TRN2", target_bir_lowering=False, debug=False,
                   enable_asserts=False, num_devices=N_CORES)

    NX = len(XDEP)
    xT_d = nc.dram_tensor("xT", [13, B_CORE], F32, kind="ExternalInput")
    W_d = nc.dram_tensor("W", [13, NX * DIM], F32, kind="ExternalInput")
    Mp_d = nc.dram_tensor("Mp", [128, 128], F16, kind="ExternalInput")
    Mn_d = nc.dram_tensor("Mn", [128, 128], F16, kind="ExternalInput")
    # G stationaries: 4 layers x 4 blocks x (GrT, GiT, nGiT) each [128,128]
    G_d = nc.dram_tensor("G", [128, 4 * 4 * 3 * 128], F16,
                         kind="ExternalInput")
    Z_d = nc.dram_tensor("Zt", [DIM, 8], F32, kind="ExternalInput")
    out_d = nc.dram_tensor("out", [B_CORE, 8], F32, kind="ExternalOutput")

    with tile.TileContext(nc) as tc:
        with (
            tc.tile_pool(name="consts", bufs=1) as consts,
            tc.tile_pool(name="psum", bufs=4, space="PSUM") as psum_pool,
            tc.tile_pool(name="ph32", bufs=3) as ph32,
            tc.tile_pool(name="cs", bufs=3) as cs_pool,
            tc.tile_pool(name="hcopy", bufs=2) as hcopy,
            tc.tile_pool(name="prods", bufs=2) as prod_pool,
            tc.tile_pool(name="state", bufs=3) as state_pool,
            tc.tile_pool(name="tail", bufs=2) as tail_pool,
        ):
            xT = consts.tile([13, B_CORE], F32R, tag="xT")
            nc.sync.dma_start(xT[:], xT_d[:].bitcast(F32R))
            Wt = consts.tile([13, NX * DIM], F32R, tag="W")
            nc.sync.dma_start(Wt[:], W_d[:].bitcast(F32R))
            Mp = consts.tile([128, 128], F16, tag="Mp")
            nc.sync.dma_start(Mp[:], Mp_d[:])
            Mn = consts.tile([128, 128], F16, tag="Mn")
            nc.sync.dma_start(Mn[:], Mn_d[:])
            Gt = consts.tile([128, 4 * 4 * 3 * 128], F16, tag="Gt")
            nc.sync.dma_start(Gt[:], G_d[:])
            Z0 = consts.tile([128, 8], F32, tag="Z0")
            nc.sync.dma_start(Z0[:], Z_d[0:128, :])
            Z1 = consts.tile([128, 8], F32, tag="Z1")
            nc.sync.dma_start(Z1[:], Z_d[128:256, :])
            hpi = consts.tile([128, 1], F32, tag="hpi")
            nc.vector.memset(hpi[:], PI / 2.0)

            def gblk(l, hp, h, ver):
                # ver: 0=GrT 1=GiT 2=nGiT
                base = ((l * 4 + hp * 2 + h) * 3 + ver) * 128
                return Gt[:, base:base + 128]

            def h_mms2(pre, pim, sre, sim):
                # both components interleaved so each stationary is loaded
                # once per (hp, h) instead of once per matmul
                for hp in (0, 1):
                    for h in (0, 1):
                        lhsT = Mn if (hp == 1 and h == 1) else Mp
                        for dst, src in ((pre, sre), (pim, sim)):
                            nc.tensor.matmul(
                                dst[:, hp * F:(hp + 1) * F],
                                lhsT[:],
                                src[:, h * F:(h + 1) * F],
                                start=(h == 0), stop=(h == 1),
                            )

            for pair in range(1):
              chunk_ids = tuple(range(NCH))
              st = {c: (None, None) for c in chunk_ids}
              xis = {c: 0 for c in chunk_ids}
              for si in range(17):
                for ch in chunk_ids:
                    bsl = slice(ch * F, (ch + 1) * F)
                    st_re, st_im = st[ch]
                    xi = xis[ch]
                    if si in CONST:
                        l = CONST[si]
                        # complex matmul G = Hu D Hu applied to state
                        pre = psum_pool.tile([128, 2 * F], F32, tag="ps")
                        pim = psum_pool.tile([128, 2 * F], F32, tag="ps")
                        for hp in (0, 1):
                            dsl = slice(hp * F, (hp + 1) * F)
                            # interleave so each Gr block is loaded once for
                            # its use in both the re and im accumulations
                            nc.tensor.matmul(pre[:, dsl], gblk(l, hp, 0, 0),
                                             st_re[:, 0:F], start=True, stop=False)
                            nc.tensor.matmul(pim[:, dsl], gblk(l, hp, 0, 0),
                                             st_im[:, 0:F], start=True, stop=False)
                            nc.tensor.matmul(pre[:, dsl], gblk(l, hp, 1, 0),
                                             st_re[:, F:2 * F], start=False, stop=False)
                            nc.tensor.matmul(pim[:, dsl], gblk(l, hp, 1, 0),
                                             st_im[:, F:2 * F], start=False, stop=False)
                            nc.tensor.matmul(pre[:, dsl], gblk(l, hp, 0, 2),
                                             st_im[:, 0:F], start=False, stop=False)
                            nc.tensor.matmul(pre[:, dsl], gblk(l, hp, 1, 2),
                                             st_im[:, F:2 * F], start=False, stop=True)
                            nc.tensor.matmul(pim[:, dsl], gblk(l, hp, 0, 1),
                                             st_re[:, 0:F], start=False, stop=False)
                            nc.tensor.matmul(pim[:, dsl], gblk(l, hp, 1, 1),
                                             st_re[:, F:2 * F], start=False, stop=True)
                        st_re = state_pool.tile([128, 2 * F], F16, tag="sre")
                        nc.scalar.activation(st_re[:], pre[:], ACTF.Copy)
                        st_im = state_pool.tile([128, 2 * F], F16, tag="sim")
                        nc.vector.tensor_copy(st_im[:], pim[:])
                        st[ch] = (st_re, st_im)
                        continue

                    kk = xi; xis[ch] = xi + 1
                    # phases: q = W_k^T x -> psum
                    qp = psum_pool.tile([128, 2 * F], F32, tag="ps")
                    base = kk * DIM
                    for h in (0, 1):
                        nc.tensor.matmul(
                            qp[:, h * F:(h + 1) * F],
                            Wt[:, base + h * 128: base + (h + 1) * 128],
                            xT[:, bsl], start=True, stop=True)
                    t = ph32.tile([128, 2 * F], F32, tag="t")
                    nc.scalar.activation(t[:], qp[:], ACTF.Copy, bias=MAGIC)
                    g = ph32.tile([128, 2 * F], F32, tag="g")
                    nc.vector.scalar_tensor_tensor(g[:], t[:], MAGIC, qp[:],
                                                   AOT.subtract, AOT.subtract)
                    S = cs_pool.tile([128, 2 * F], F16, tag="S")
                    nc.scalar.activation(S[:], g[:], ACTF.Sin, scale=-2.0 * PI)
                    a = ph32.tile([128, 2 * F], F32, tag="a")
                    nc.scalar.activation(a[:], g[:], ACTF.Abs)
                    C = cs_pool.tile([128, 2 * F], F16, tag="C")
                    nc.scalar.activation(C[:], a[:], ACTF.Sin, bias=hpi[:],
                                         scale=-2.0 * PI)
                    if si == 0:
                        st[ch] = (C, S)
                        continue

                    if (si - 1) in CONST:
                        # preceding merged G already applied the trailing H:
                        # modulate the SBUF f16 state directly
                        reh, imh = st_re, st_im
                    else:
                        pre = psum_pool.tile([128, 2 * F], F32, tag="ps")
                        pim = psum_pool.tile([128, 2 * F], F32, tag="ps")
                        h_mms2(pre, pim, st_re, st_im)
                        reh = hcopy.tile([128, 2 * F], F16, tag="reh")
                        nc.scalar.activation(reh[:], pre[:], ACTF.Copy)
                        imh = hcopy.tile([128, 2 * F], F16, tag="imh")
                        nc.vector.tensor_copy(imh[:], pim[:])

                    p_rc = prod_pool.tile([128, 2 * F], F16, tag="prc")
                    nc.vector.tensor_mul(p_rc[:], reh[:], C[:])
                    p_is = prod_pool.tile([128, 2 * F], F16, tag="pis")
                    nc.vector.tensor_mul(p_is[:], imh[:], S[:])
                    p_rs = prod_pool.tile([128, 2 * F], F16, tag="prs")
                    nc.vector.tensor_mul(p_rs[:], reh[:], S[:])
                    p_ic = prod_pool.tile([128, 2 * F], F16, tag="pic")
                    nc.vector.tensor_mul(p_ic[:], imh[:], C[:])
                    st_re = state_pool.tile([128, 2 * F], F16, tag="sre")
                    nc.vector.tensor_sub(st_re[:], p_rc[:], p_is[:])
                    st_im = state_pool.tile([128, 2 * F], F16, tag="sim")
                    nc.vector.tensor_add(st_im[:], p_rs[:], p_ic[:])
                    st[ch] = (st_re, st_im)

              # final H, probs, Z projection
              for ch in chunk_ids:
                st_re, st_im = st[ch]
                pre = psum_pool.tile([128, 2 * F], F32, tag="ps")
                pim = psum_pool.tile([128, 2 * F], F32, tag="ps")
                h_mms2(pre, pim, st_re, st_im)
                p1 = tail_pool.tile([128, 2 * F], F32, tag="p1")
                nc.scalar.activation(p1[:], pre[:], ACTF.Square, scale=1.0 / 16.0)
                p2 = tail_pool.tile([128, 2 * F], F32, tag="p2")
                nc.scalar.activation(p2[:], pim[:], ACTF.Square, scale=1.0 / 16.0)
                probs = tail_pool.tile([128, 2 * F], F32, tag="probs")
                nc.gpsimd.tensor_add(probs[:], p1[:], p2[:])
                for sub in range(F // 128):
                    zp = psum_pool.tile([128, 8], F32, tag="ps")
                    nc.tensor.matmul(zp[:], probs[:, sub * 128:(sub + 1) * 128],
                                     Z0[:], start=True, stop=False)
                    nc.tensor.matmul(zp[:], probs[:, F + sub * 128: F + (sub + 1) * 128],
                                     Z1[:], start=False, stop=True)
                    zs = tail_pool.tile([128, 8], F32, tag="zs")
                    nc.scalar.activation(zs[:], zp[:], ACTF.Copy)
                    nc.sync.dma_start(
                        out_d[ch * F + sub * 128: ch * F + (sub + 1) * 128, :],
                        zs[:])
    _legalize_waits(nc)
    return nc


_PROGRAM_CACHE = {}


def kernel(observation, theta, omega, _trace=False):
    observation = np.asarray(observation, np.float32)
    W, M, G_blocks, Z = _build_host_tables(theta, omega)
    NX = len(XDEP)
    W_flat = np.ascontiguousarray(
        W.transpose(1, 0, 2).reshape(13, NX * DIM))
    x_augT = np.concatenate(
        [observation, np.ones((B_TOTAL, 1), np.float32)], 1).T  # (13, 16384)
    x_augT = np.ascontiguousarray(x_augT)

    G_flat = np.zeros((128, 4 * 4 * 3 * 128), np.float32)
    for l in range(4):
        for b in range(4):     # hp*2+h
            GrT, GiT = G_blocks[l][b]
            base = ((l * 4 + b) * 3) * 128
            G_flat[:, base:base + 128] = GrT
            G_flat[:, base + 128:base + 256] = GiT
            G_flat[:, base + 256:base + 384] = -GiT

    if "nc" not in _PROGRAM_CACHE:
        _PROGRAM_CACHE["nc"] = _build_program()
    nc = _PROGRAM_CACHE["nc"]

    in_maps = []
    for c in range(N_CORES):
        in_maps.append({
            "xT": np.ascontiguousarray(x_augT[:, c * B_CORE:(c + 1) * B_CORE]),
            "W": W_flat,
            "Mp": M.astype(np.float16),
            "Mn": np.ascontiguousarray(-M).astype(np.float16),
            "G": G_flat.astype(np.float16),
            "Zt": Z,
        })
    res = run_bass_kernel_spmd(nc, in_maps, core_ids=list(range(N_CORES)),
                               trace=_trace)
    out = np.concatenate([r["out"] for r in res.results], 0)
    if _trace:
        kernel.last_results = res
    return out


# revision 5
# speedup vs baseline: 1.0299x; 1.0299x over previous
"""Trainium2 Bass kernel v2 for nn_DataReuploadingTorso.

Circuit = 17 diagonal phase layers D_k interleaved with H^{x8}. Four of the
D_k (the theta1-only layers) are batch-independent: they are folded into
dense complex matrices G_l = H D H on the host, applied as f16 PE matmuls
with PSUM-accumulated real/imag parts (no elementwise work). The remaining
13 x-dependent layers use: q = W^T x (fp32r PE), magic-number rne range
reduction (ACT copy-bias + DVE STT), S = sin(2*pi*q), C = cos via
Sin(pi/2 - 2*pi*|g|), then an f16 complex modulation split across
ACT/DVE/Pool.

Sharding: pure data parallel over batch, 8 cores x 2048.
State layout: [128 partitions = s mod 128, (s_hi half | batch)] packed
[128, 2F], f16.
"""
import numpy as np

import concourse.bass as bass
import concourse.mybir as mybir
import concourse.tile as tile
from concourse.bass_utils import run_bass_kernel_spmd

N_CORES = 8
B_TOTAL = 16384
B_CORE = B_TOTAL // N_CORES      # 2048
F = 512                          # batch per chunk
NCH = B_CORE // F                # 4 chunks
DIM = 256
N_Q = 8

F32 = mybir.dt.float32
F32R = mybir.dt.float32r
F16 = mybir.dt.float16
AOT = mybir.AluOpType
ACTF = mybir.ActivationFunctionType

PI = float(np.pi)
MAGIC = float(np.float32(1.5 * 2 ** 23))

XDEP = [0, 1, 3, 4, 5, 7, 8, 9, 11, 12, 13, 15, 16]   # x-dependent layers
CONST = {2: 0, 6: 1, 10: 2, 14: 3}                     # layer idx -> G idx
FUSE_ADD = {3, 4, 7, 8, 11, 12, 15, 16}  # st_im add folded into next H-app


# ----------------------------------------------------------------- host tables
def _build_host_tables(theta, omega):
    theta = np.asarray(theta, np.float64)              # (8, 5, 3)
    omega = np.asarray(omega, np.float64).reshape(5, 8, 3)

    idx = np.arange(DIM)
    beta = np.stack([(idx >> (7 - q)) & 1 for q in range(N_Q)], 0)   # (8, 256)
    sgn = (2 * beta - 1).astype(np.float64)

    def czterm(pairs):
        t = np.zeros(DIM)
        for a, b in pairs:
            t += np.pi * (beta[a] * beta[b])
        return t
    cz_even = czterm([(0, 1), (2, 3), (4, 5), (6, 7)])
    cz_odd = czterm([(1, 2), (3, 4), (5, 6)])

    steps = []
    for l in range(4):
        A = np.zeros((12, 8)); k = np.zeros(8)
        for q in range(8):
            A[3 * (q % 4) + 1, q] = omega[l, q, 1]
        steps.append((A, k, None))
        A = np.zeros((12, 8)); k = np.zeros(8)
        for q in range(8):
            A[3 * (q % 4) + 2, q] = omega[l, q, 2]
            k[q] = theta[q, l, 0]
        steps.append((A, k, None))
        A = np.zeros((12, 8)); k = theta[:, l, 1].copy()
        steps.append((A, k, None))
        A = np.zeros((12, 8)); k = theta[:, l, 2].copy()
        for q in range(8):
            A[3 * (q % 4) + 0, q] = omega[l + 1, q, 0]
        if l + 1 == 4:
            k += theta[:, 4, 0]
        steps.append((A, k, cz_even if l % 2 == 0 else cz_odd))
    A = np.zeros((12, 8)); k = theta[:, 4, 1].copy()
    for q in range(8):
        A[3 * (q % 4) + 1, q] = omega[4, q, 1]
    steps.append((A, k, None))

    inv2pi = 1.0 / (2.0 * np.pi)
    W = np.zeros((len(XDEP), 13, DIM))
    for i, si in enumerate(XDEP):
        A, k, cz = steps[si]
        W[i, :12] = (A @ (sgn * 0.5)) * inv2pi
        c = k @ (sgn * 0.5)
        if cz is not None:
            c = c + cz
        W[i, 12] = (np.mod(c + np.pi, 2 * np.pi) - np.pi) * inv2pi

    # H^{x8} sign matrix and unitary
    sp = np.arange(DIM)
    pop = np.zeros((DIM, DIM), np.int64)
    for q in range(8):
        pop += np.outer((sp >> q) & 1, (sp >> q) & 1)
    H8 = np.where(pop % 2 == 0, 1.0, -1.0)
    Hu = H8 / 16.0                                     # exact unitary
    M = (H8[:128, :128] / 16.0).astype(np.float32)     # H^{x7}sgn / 16

    # merged G_l = Hu diag(e^{i Phi_c}) Hu for the theta1-only layers
    G_blocks = []                                      # (4, hp, h) -> (GrT, GiT)
    for si in sorted(CONST):
        _, k, _ = steps[si]
        phic = k @ (sgn * 0.5)                         # (256,)
        G = (Hu * np.exp(1j * phic)[None, :]) @ Hu     # Hu @ diag @ Hu
        blocks = []
        for hp in range(2):
            for h in range(2):
                B = G[hp * 128:(hp + 1) * 128, h * 128:(h + 1) * 128]
                blocks.append((B.real.T.copy(), B.imag.T.copy()))
        G_blocks.append(blocks)

    Z = np.stack([1.0 - 2.0 * ((idx >> (7 - q)) & 1) for q in range(8)], 1)
    return W.astype(np.float32), M, G_blocks, Z.astype(np.float32)


# -------------------------------------------------------------- device program
def _legalize_waits(nc, limit=1):
    """walrus codegen allows only one embedded sync-wait on several TRN2
    instruction encodings. Hoist excess waits onto same-engine NoOps."""
    def fix_block(blk):
        new_insts = []
        for ins in blk.instructions:
            si = getattr(ins, "sync_info", None)
            waits = list(si.on_wait) if si and si.on_wait else []
            if len(waits) > limit:
                keep = waits[-limit:]
                for j, w in enumerate(waits[:-limit]):
                    new_insts.append(mybir.InstNoOp(
                        name=f"{ins.name}-w{j}",
                        engine=ins.engine,
                        sync_info=mybir.SyncInfo(on_wait=[w], on_update=[]),
                    ))
                si.on_wait = keep
            new_insts.append(ins)
        blk.instructions = new_insts
        for sb in getattr(blk, "blocks", None) or []:
            fix_block(sb)
    for f in nc.m.functions:
        for blk in f.blocks:
            fix_block(blk)


def _build_program():
    nc = bass.Bass("TRN2", target_bir_lowering=False, debug=False,
                   enable_asserts=False, num_devices=N_CORES)

    NX = len(XDEP)
    xT_d = nc.dram_tensor("xT", [13, B_CORE], F32, kind="ExternalInput")
    W_d = nc.dram_tensor("W", [13, NX * DIM], F32, kind="ExternalInput")
    Mp_d = nc.dram_tensor("Mp", [128, 128], F16, kind="ExternalInput")
    Mn_d = nc.dram_tensor("Mn", [128, 128], F16, kind="ExternalInput")
    # G stationaries: 4 layers x 4 blocks x (GrT, GiT, nGiT) each [128,128]
    G_d = nc.dram_tensor("G", [128, 4 * 4 * 3 * 128], F16,
                         kind="ExternalInput")
    Z_d = nc.dram_tensor("Zt", [DIM, 8], F32, kind="ExternalInput")
    out_d = nc.dram_tensor("out", [B_CORE, 8], F32, kind="ExternalOutput")

    with tile.TileContext(nc) as tc:
        with (
            tc.tile_pool(name="consts", bufs=1) as consts,
            tc.tile_pool(name="psum", bufs=4, space="PSUM") as psum_pool,
            tc.tile_pool(name="ph32", bufs=3) as ph32,
            tc.tile_pool(name="cs", bufs=3) as cs_pool,
            tc.tile_pool(name="hcopy", bufs=2) as hcopy,
            tc.tile_pool(name="prods", bufs=2) as prod_pool,
            tc.tile_pool(name="fprod", bufs=5) as fprod_pool,
            tc.tile_pool(name="state", bufs=3) as state_pool,
            tc.tile_pool(name="tail", bufs=2) as tail_pool,
        ):
            xT = consts.tile([13, B_CORE], F32R, tag="xT")
            nc.sync.dma_start(xT[:], xT_d[:].bitcast(F32R))
            Wt = consts.tile([13, NX * DIM], F32R, tag="W")
            nc.sync.dma_start(Wt[:], W_d[:].bitcast(F32R))
            Mp = consts.tile([128, 128], F16, tag="Mp")
            nc.sync.dma_start(Mp[:], Mp_d[:])
            Mn = consts.tile([128, 128], F16, tag="Mn")
            nc.sync.dma_start(Mn[:], Mn_d[:])
            Gt = consts.tile([128, 4 * 4 * 3 * 128], F16, tag="Gt")
            nc.sync.dma_start(Gt[:], G_d[:])
            Z0 = consts.tile([128, 8], F32, tag="Z0")
            nc.sync.dma_start(Z0[:], Z_d[0:128, :])
            Z1 = consts.tile([128, 8], F32, tag="Z1")
            nc.sync.dma_start(Z1[:], Z_d[128:256, :])
            hpi = consts.tile([128, 1], F32, tag="hpi")
            nc.vector.memset(hpi[:], PI / 2.0)

            def gblk(l, hp, h, ver):
                # ver: 0=GrT 1=GiT 2=nGiT
                base = ((l * 4 + hp * 2 + h) * 3 + ver) * 128
                return Gt[:, base:base + 128]

            def h_mms2(pre, pim, sre, sim):
                # sim is either a materialized f16 state or a (p_rs, p_ic)
                # pair whose sum is folded into the im accumulation groups
                sims = sim if isinstance(sim, tuple) else (sim,)
                for hp in (0, 1):
                    for h in (0, 1):
                        lhsT = Mn if (hp == 1 and h == 1) else Mp
                        nc.tensor.matmul(
                            pre[:, hp * F:(hp + 1) * F], lhsT[:],
                            sre[:, h * F:(h + 1) * F],
                            start=(h == 0), stop=(h == 1))
                        for j, s in enumerate(sims):
                            nc.tensor.matmul(
                                pim[:, hp * F:(hp + 1) * F], lhsT[:],
                                s[:, h * F:(h + 1) * F],
                                start=(h == 0 and j == 0),
                                stop=(h == 1 and j == len(sims) - 1))

            for pair in range(1):
              chunk_ids = tuple(range(NCH))
              st = {c: (None, None) for c in chunk_ids}
              xis = {c: 0 for c in chunk_ids}
              for si in range(17):
                for ch in chunk_ids:
                    bsl = slice(ch * F, (ch + 1) * F)
                    st_re, st_im = st[ch]
                    xi = xis[ch]
                    if si in CONST:
                        l = CONST[si]
                        # complex matmul G = Hu D Hu applied to state
                        pre = psum_pool.tile([128, 2 * F], F32, tag="ps")
                        pim = psum_pool.tile([128, 2 * F], F32, tag="ps")
                        for hp in (0, 1):
                            dsl = slice(hp * F, (hp + 1) * F)
                            # interleave so each Gr block is loaded once for
                            # its use in both the re and im accumulations
                            nc.tensor.matmul(pre[:, dsl], gblk(l, hp, 0, 0),
                                             st_re[:, 0:F], start=True, stop=False)
                            nc.tensor.matmul(pim[:, dsl], gblk(l, hp, 0, 0),
                                             st_im[:, 0:F], start=True, stop=False)
                            nc.tensor.matmul(pre[:, dsl], gblk(l, hp, 1, 0),
                                             st_re[:, F:2 * F], start=False, stop=False)
                            nc.tensor.matmul(pim[:, dsl], gblk(l, hp, 1, 0),
                                             st_im[:, F:2 * F], start=False, stop=False)
                            nc.tensor.matmul(pre[:, dsl], gblk(l, hp, 0, 2),
                                             st_im[:, 0:F], start=False, stop=False)
                            nc.tensor.matmul(pre[:, dsl], gblk(l, hp, 1, 2),
                                             st_im[:, F:2 * F], start=False, stop=True)
                            nc.tensor.matmul(pim[:, dsl], gblk(l, hp, 0, 1),
                                             st_re[:, 0:F], start=False, stop=False)
                            nc.tensor.matmul(pim[:, dsl], gblk(l, hp, 1, 1),
                                             st_re[:, F:2 * F], start=False, stop=True)
                        st_re = state_pool.tile([128, 2 * F], F16, tag="sre")
                        nc.scalar.activation(st_re[:], pre[:], ACTF.Copy)
                        st_im = state_pool.tile([128, 2 * F], F16, tag="sim")
                        nc.vector.tensor_copy(st_im[:], pim[:])
                        st[ch] = (st_re, st_im)
                        continue

                    kk = xi; xis[ch] = xi + 1
                    # phases: q = W_k^T x -> psum
                    qp = psum_pool.tile([128, 2 * F], F32, tag="ps")
                    base = kk * DIM
                    for h in (0, 1):
                        nc.tensor.matmul(
                            qp[:, h * F:(h + 1) * F],
                            Wt[:, base + h * 128: base + (h + 1) * 128],
                            xT[:, bsl], start=True, stop=True)
                    t = ph32.tile([128, 2 * F], F32, tag="t")
                    nc.scalar.activation(t[:], qp[:], ACTF.Copy, bias=MAGIC)
                    g = ph32.tile([128, 2 * F], F32, tag="g")
                    nc.vector.scalar_tensor_tensor(g[:], t[:], MAGIC, qp[:],
                                                   AOT.subtract, AOT.subtract)
                    S = cs_pool.tile([128, 2 * F], F16, tag="S")
                    nc.scalar.activation(S[:], g[:], ACTF.Sin, scale=-2.0 * PI)
                    a = ph32.tile([128, 2 * F], F32, tag="a")
                    nc.scalar.activation(a[:], g[:], ACTF.Abs)
                    C = cs_pool.tile([128, 2 * F], F16, tag="C")
                    nc.scalar.activation(C[:], a[:], ACTF.Sin, bias=hpi[:],
                                         scale=-2.0 * PI)
                    if si == 0:
                        st[ch] = (C, S)
                        continue

                    if (si - 1) in CONST:
                        # preceding merged G already applied the trailing H:
                        # modulate the SBUF f16 state directly
                        reh, imh = st_re, st_im
                    else:
                        pre = psum_pool.tile([128, 2 * F], F32, tag="ps")
                        pim = psum_pool.tile([128, 2 * F], F32, tag="ps")
                        h_mms2(pre, pim, st_re, st_im)
                        reh = hcopy.tile([128, 2 * F], F16, tag="reh")
                        nc.scalar.activation(reh[:], pre[:], ACTF.Copy)
                        imh = hcopy.tile([128, 2 * F], F16, tag="imh")
                        nc.vector.tensor_copy(imh[:], pim[:])

                    p_rc = prod_pool.tile([128, 2 * F], F16, tag="prc")
                    nc.vector.tensor_mul(p_rc[:], reh[:], C[:])
                    p_is = prod_pool.tile([128, 2 * F], F16, tag="pis")
                    nc.vector.tensor_mul(p_is[:], imh[:], S[:])
                    p_rs = fprod_pool.tile([128, 2 * F], F16, tag="prs")
                    nc.vector.tensor_mul(p_rs[:], reh[:], S[:])
                    p_ic = fprod_pool.tile([128, 2 * F], F16, tag="pic")
                    nc.vector.tensor_mul(p_ic[:], imh[:], C[:])
                    st_re = state_pool.tile([128, 2 * F], F16, tag="sre")
                    nc.vector.tensor_sub(st_re[:], p_rc[:], p_is[:])
                    if si in FUSE_ADD:
                        # st_im = p_rs + p_ic folded into the next H-app's
                        # PSUM accumulation (consumer is always an H, not G)
                        st[ch] = (st_re, (p_rs, p_ic))
                    else:
                        st_im = state_pool.tile([128, 2 * F], F16, tag="sim")
                        nc.vector.tensor_add(st_im[:], p_rs[:], p_ic[:])
                        st[ch] = (st_re, st_im)

              # final H, probs, Z projection
              for ch in chunk_ids:
                st_re, st_im = st[ch]
                pre = psum_pool.tile([128, 2 * F], F32, tag="ps")
                pim = psum_pool.tile([128, 2 * F], F32, tag="ps")
                h_mms2(pre, pim, st_re, st_im)
                p1 = tail_pool.tile([128, 2 * F], F32, tag="p1")
                nc.scalar.activation(p1[:], pre[:], ACTF.Square, scale=1.0 / 16.0)
                p2 = tail_pool.tile([128, 2 * F], F32, tag="p2")
                nc.scalar.activation(p2[:], pim[:], ACTF.Square, scale=1.0 / 16.0)
                probs = tail_pool.tile([128, 2 * F], F32, tag="probs")
                nc.gpsimd.tensor_add(probs[:], p1[:], p2[:])
                for sub in range(F // 128):
                    zp = psum_pool.tile([128, 8], F32, tag="ps")
                    nc.tensor.matmul(zp[:], probs[:, sub * 128:(sub + 1) * 128],
                                     Z0[:], start=True, stop=False)
                    nc.tensor.matmul(zp[:], probs[:, F + sub * 128: F + (sub + 1) * 128],
                                     Z1[:], start=False, stop=True)
                    zs = tail_pool.tile([128, 8], F32, tag="zs")
                    nc.scalar.activation(zs[:], zp[:], ACTF.Copy)
                    nc.sync.dma_start(
                        out_d[ch * F + sub * 128: ch * F + (sub + 1) * 128, :],
                        zs[:])
    _legalize_waits(nc)
    return nc


_PROGRAM_CACHE = {}


def kernel(observation, theta, omega, _trace=False):
    observation = np.asarray(observation, np.float32)
    W, M, G_blocks, Z = _build_host_tables(theta, omega)
    NX = len(XDEP)
    W_flat = np.ascontiguousarray(
        W.transpose(1, 0, 2).reshape(13, NX * DIM))
    x_augT = np.concatenate(
        [observation, np.ones((B_TOTAL, 1), np.float32)], 1).T  # (13, 16384)
    x_augT = np.ascontiguousarray(x_augT)

    G_flat = np.zeros((128, 4 * 4 * 3 * 128), np.float32)
    for l in range(4):
        for b in range(4):     # hp*2+h
            GrT, GiT = G_blocks[l][b]
            base = ((l * 4 + b) * 3) * 128
            G_flat[:, base:base + 128] = GrT
            G_flat[:, base + 128:base + 256] = GiT
            G_flat[:, base + 256:base + 384] = -GiT

    if "nc" not in _PROGRAM_CACHE:
        _PROGRAM_CACHE["nc"] = _build_program()
    nc = _PROGRAM_CACHE["nc"]

    in_maps = []
    for c in range(N_CORES):
        in_maps.append({
            "xT": np.ascontiguousarray(x_augT[:, c * B_CORE:(c + 1) * B_CORE]),
            "W": W_flat,
            "Mp": M.astype(np.float16),
            "Mn": np.ascontiguousarray(-M).astype(np.float16),
            "G": G_flat.astype(np.float16),
            "Zt": Z,
        })
    res = run_bass_kernel_spmd(nc, in_maps, core_ids=list(range(N_CORES)),
                               trace=_trace)
    out = np.concatenate([r["out"] for r in res.results], 0)
    if _trace:
        kernel.last_results = res
    return out


# revision 6
# speedup vs baseline: 1.0511x; 1.0206x over previous
"""Trainium2 Bass kernel v2 for nn_DataReuploadingTorso.

Circuit = 17 diagonal phase layers D_k interleaved with H^{x8}. Four of the
D_k (the theta1-only layers) are batch-independent: they are folded into
dense complex matrices G_l = H D H on the host, applied as f16 PE matmuls
with PSUM-accumulated real/imag parts (no elementwise work). The remaining
13 x-dependent layers use: q = W^T x (fp32r PE), magic-number rne range
reduction (ACT copy-bias + DVE STT), S = sin(2*pi*q), C = cos via
Sin(pi/2 - 2*pi*|g|), then an f16 complex modulation split across
ACT/DVE/Pool.

Sharding: pure data parallel over batch, 8 cores x 2048.
State layout: [128 partitions = s mod 128, (s_hi half | batch)] packed
[128, 2F], f16.
"""
import numpy as np

import concourse.bass as bass
import concourse.mybir as mybir
import concourse.tile as tile
from concourse.bass_utils import run_bass_kernel_spmd

N_CORES = 8
B_TOTAL = 16384
B_CORE = B_TOTAL // N_CORES      # 2048
F = 512                          # batch per chunk
NCH = B_CORE // F                # 4 chunks
DIM = 256
N_Q = 8

F32 = mybir.dt.float32
F32R = mybir.dt.float32r
F16 = mybir.dt.float16
AOT = mybir.AluOpType
ACTF = mybir.ActivationFunctionType

PI = float(np.pi)
MAGIC = float(np.float32(1.5 * 2 ** 23))

XDEP = [0, 1, 3, 4, 5, 7, 8, 9, 11, 12, 13, 15, 16]   # x-dependent layers
CONST = {2: 0, 6: 1, 10: 2, 14: 3}                     # layer idx -> G idx
FUSE_ADD = {3, 4, 7, 8, 11, 12, 15, 16}  # st_im add folded into next H-app


# ----------------------------------------------------------------- host tables
def _build_host_tables(theta, omega):
    theta = np.asarray(theta, np.float64)              # (8, 5, 3)
    omega = np.asarray(omega, np.float64).reshape(5, 8, 3)

    idx = np.arange(DIM)
    beta = np.stack([(idx >> (7 - q)) & 1 for q in range(N_Q)], 0)   # (8, 256)
    sgn = (2 * beta - 1).astype(np.float64)

    def czterm(pairs):
        t = np.zeros(DIM)
        for a, b in pairs:
            t += np.pi * (beta[a] * beta[b])
        return t
    cz_even = czterm([(0, 1), (2, 3), (4, 5), (6, 7)])
    cz_odd = czterm([(1, 2), (3, 4), (5, 6)])

    steps = []
    for l in range(4):
        A = np.zeros((12, 8)); k = np.zeros(8)
        for q in range(8):
            A[3 * (q % 4) + 1, q] = omega[l, q, 1]
        steps.append((A, k, None))
        A = np.zeros((12, 8)); k = np.zeros(8)
        for q in range(8):
            A[3 * (q % 4) + 2, q] = omega[l, q, 2]
            k[q] = theta[q, l, 0]
        steps.append((A, k, None))
        A = np.zeros((12, 8)); k = theta[:, l, 1].copy()
        steps.append((A, k, None))
        A = np.zeros((12, 8)); k = theta[:, l, 2].copy()
        for q in range(8):
            A[3 * (q % 4) + 0, q] = omega[l + 1, q, 0]
        if l + 1 == 4:
            k += theta[:, 4, 0]
        steps.append((A, k, cz_even if l % 2 == 0 else cz_odd))
    A = np.zeros((12, 8)); k = theta[:, 4, 1].copy()
    for q in range(8):
        A[3 * (q % 4) + 1, q] = omega[4, q, 1]
    steps.append((A, k, None))

    inv2pi = 1.0 / (2.0 * np.pi)
    W = np.zeros((len(XDEP), 13, DIM))
    for i, si in enumerate(XDEP):
        A, k, cz = steps[si]
        W[i, :12] = (A @ (sgn * 0.5)) * inv2pi
        c = k @ (sgn * 0.5)
        if cz is not None:
            c = c + cz
        W[i, 12] = (np.mod(c + np.pi, 2 * np.pi) - np.pi) * inv2pi

    # H^{x8} sign matrix and unitary
    sp = np.arange(DIM)
    pop = np.zeros((DIM, DIM), np.int64)
    for q in range(8):
        pop += np.outer((sp >> q) & 1, (sp >> q) & 1)
    H8 = np.where(pop % 2 == 0, 1.0, -1.0)
    Hu = H8 / 16.0                                     # exact unitary
    M = (H8[:128, :128] / 16.0).astype(np.float32)     # H^{x7}sgn / 16

    # merged G_l = Hu diag(e^{i Phi_c}) Hu for the theta1-only layers
    G_blocks = []                                      # (4, hp, h) -> (GrT, GiT)
    for si in sorted(CONST):
        _, k, _ = steps[si]
        phic = k @ (sgn * 0.5)                         # (256,)
        G = (Hu * np.exp(1j * phic)[None, :]) @ Hu     # Hu @ diag @ Hu
        blocks = []
        for hp in range(2):
            for h in range(2):
                B = G[hp * 128:(hp + 1) * 128, h * 128:(h + 1) * 128]
                blocks.append((B.real.T.copy(), B.imag.T.copy()))
        G_blocks.append(blocks)

    Z = np.stack([1.0 - 2.0 * ((idx >> (7 - q)) & 1) for q in range(8)], 1)
    return W.astype(np.float32), M, G_blocks, Z.astype(np.float32)


# -------------------------------------------------------------- device program
def _legalize_waits(nc, limit=1):
    """walrus codegen allows only one embedded sync-wait on several TRN2
    instruction encodings. Hoist excess waits onto same-engine NoOps."""
    def fix_block(blk):
        new_insts = []
        for ins in blk.instructions:
            si = getattr(ins, "sync_info", None)
            waits = list(si.on_wait) if si and si.on_wait else []
            if len(waits) > limit:
                keep = waits[-limit:]
                for j, w in enumerate(waits[:-limit]):
                    new_insts.append(mybir.InstNoOp(
                        name=f"{ins.name}-w{j}",
                        engine=ins.engine,
                        sync_info=mybir.SyncInfo(on_wait=[w], on_update=[]),
                    ))
                si.on_wait = keep
            new_insts.append(ins)
        blk.instructions = new_insts
        for sb in getattr(blk, "blocks", None) or []:
            fix_block(sb)
    for f in nc.m.functions:
        for blk in f.blocks:
            fix_block(blk)


def _build_program():
    nc = bass.Bass("TRN2", target_bir_lowering=False, debug=False,
                   enable_asserts=False, num_devices=N_CORES)

    NX = len(XDEP)
    xT_d = nc.dram_tensor("xT", [13, B_CORE], F32, kind="ExternalInput")
    W_d = nc.dram_tensor("W", [13, NX * DIM], F32, kind="ExternalInput")
    Mp_d = nc.dram_tensor("Mp", [128, 128], F16, kind="ExternalInput")
    Mn_d = nc.dram_tensor("Mn", [128, 128], F16, kind="ExternalInput")
    # G stationaries: 4 layers x 4 blocks x (GrT, GiT, nGiT) each [128,128]
    G_d = nc.dram_tensor("G", [128, 4 * 4 * 3 * 128], F16,
                         kind="ExternalInput")
    Z_d = nc.dram_tensor("Zt", [DIM, 8], F32, kind="ExternalInput")
    out_d = nc.dram_tensor("out", [B_CORE, 8], F32, kind="ExternalOutput")

    with tile.TileContext(nc) as tc:
        with (
            tc.tile_pool(name="consts", bufs=1) as consts,
            tc.tile_pool(name="psum", bufs=4, space="PSUM") as psum_pool,
            tc.tile_pool(name="ph32", bufs=3) as ph32,
            tc.tile_pool(name="cs", bufs=3) as cs_pool,
            tc.tile_pool(name="hcopy", bufs=2) as hcopy,
            tc.tile_pool(name="prods", bufs=2) as prod_pool,
            tc.tile_pool(name="fprod", bufs=5) as fprod_pool,
            tc.tile_pool(name="state", bufs=3) as state_pool,
            tc.tile_pool(name="tail", bufs=2) as tail_pool,
        ):
            xT = consts.tile([13, B_CORE], F32R, tag="xT")
            nc.sync.dma_start(xT[:], xT_d[:].bitcast(F32R))
            Wt = consts.tile([13, NX * DIM], F32R, tag="W")
            nc.sync.dma_start(Wt[:], W_d[:].bitcast(F32R))
            Mp = consts.tile([128, 128], F16, tag="Mp")
            nc.sync.dma_start(Mp[:], Mp_d[:])
            Mn = consts.tile([128, 128], F16, tag="Mn")
            nc.sync.dma_start(Mn[:], Mn_d[:])
            Gt = consts.tile([128, 4 * 4 * 3 * 128], F16, tag="Gt")
            nc.sync.dma_start(Gt[:], G_d[:])
            Z0 = consts.tile([128, 8], F32, tag="Z0")
            nc.sync.dma_start(Z0[:], Z_d[0:128, :])
            Z1 = consts.tile([128, 8], F32, tag="Z1")
            nc.sync.dma_start(Z1[:], Z_d[128:256, :])
            hpi = consts.tile([128, 1], F32, tag="hpi")
            nc.vector.memset(hpi[:], PI / 2.0)

            def gblk(l, hp, h, ver):
                # ver: 0=GrT 1=GiT 2=nGiT
                base = ((l * 4 + hp * 2 + h) * 3 + ver) * 128
                return Gt[:, base:base + 128]

            def h_mms2(pre, pim, sre, sim):
                # sim is either a materialized f16 state or a (p_rs, p_ic)
                # pair whose sum is folded into the im accumulation groups
                sims = sim if isinstance(sim, tuple) else (sim,)
                for hp in (0, 1):
                    for h in (0, 1):
                        lhsT = Mn if (hp == 1 and h == 1) else Mp
                        nc.tensor.matmul(
                            pre[:, hp * F:(hp + 1) * F], lhsT[:],
                            sre[:, h * F:(h + 1) * F],
                            start=(h == 0), stop=(h == 1))
                        for j, s in enumerate(sims):
                            nc.tensor.matmul(
                                pim[:, hp * F:(hp + 1) * F], lhsT[:],
                                s[:, h * F:(h + 1) * F],
                                start=(h == 0 and j == 0),
                                stop=(h == 1 and j == len(sims) - 1))

            for pair in range(1):
              chunk_ids = tuple(range(NCH))
              st = {c: (None, None) for c in chunk_ids}
              xis = {c: 0 for c in chunk_ids}
              for si in range(17):
                for ch in chunk_ids:
                    bsl = slice(ch * F, (ch + 1) * F)
                    st_re, st_im = st[ch]
                    xi = xis[ch]
                    if si in CONST:
                        l = CONST[si]
                        # complex matmul G = Hu D Hu applied to state
                        pre = psum_pool.tile([128, 2 * F], F32, tag="ps")
                        pim = psum_pool.tile([128, 2 * F], F32, tag="ps")
                        for hp in (0, 1):
                            dsl = slice(hp * F, (hp + 1) * F)
                            # st_re-consuming matmuls for both components
                            # first: st_im (added on Pool) arrives ~8 matmuls
                            # late without stalling the PE
                            nc.tensor.matmul(pre[:, dsl], gblk(l, hp, 0, 0),
                                             st_re[:, 0:F], start=True, stop=False)
                            nc.tensor.matmul(pre[:, dsl], gblk(l, hp, 1, 0),
                                             st_re[:, F:2 * F], start=False, stop=False)
                            nc.tensor.matmul(pim[:, dsl], gblk(l, hp, 0, 1),
                                             st_re[:, 0:F], start=True, stop=False)
                            nc.tensor.matmul(pim[:, dsl], gblk(l, hp, 1, 1),
                                             st_re[:, F:2 * F], start=False, stop=False)
                        for hp in (0, 1):
                            dsl = slice(hp * F, (hp + 1) * F)
                            nc.tensor.matmul(pre[:, dsl], gblk(l, hp, 0, 2),
                                             st_im[:, 0:F], start=False, stop=False)
                            nc.tensor.matmul(pre[:, dsl], gblk(l, hp, 1, 2),
                                             st_im[:, F:2 * F], start=False, stop=True)
                            nc.tensor.matmul(pim[:, dsl], gblk(l, hp, 0, 0),
                                             st_im[:, 0:F], start=False, stop=False)
                            nc.tensor.matmul(pim[:, dsl], gblk(l, hp, 1, 0),
                                             st_im[:, F:2 * F], start=False, stop=True)
                        st_re = state_pool.tile([128, 2 * F], F16, tag="sre")
                        nc.scalar.activation(st_re[:], pre[:], ACTF.Copy)
                        st_im = state_pool.tile([128, 2 * F], F16, tag="sim")
                        nc.vector.tensor_copy(st_im[:], pim[:])
                        st[ch] = (st_re, st_im)
                        continue

                    kk = xi; xis[ch] = xi + 1
                    # phases: q = W_k^T x -> psum
                    qp = psum_pool.tile([128, 2 * F], F32, tag="ps")
                    base = kk * DIM
                    for h in (0, 1):
                        nc.tensor.matmul(
                            qp[:, h * F:(h + 1) * F],
                            Wt[:, base + h * 128: base + (h + 1) * 128],
                            xT[:, bsl], start=True, stop=True)
                    t = ph32.tile([128, 2 * F], F32, tag="t")
                    nc.scalar.activation(t[:], qp[:], ACTF.Copy, bias=MAGIC)
                    g = ph32.tile([128, 2 * F], F32, tag="g")
                    nc.vector.scalar_tensor_tensor(g[:], t[:], MAGIC, qp[:],
                                                   AOT.subtract, AOT.subtract)
                    S = cs_pool.tile([128, 2 * F], F16, tag="S")
                    nc.scalar.activation(S[:], g[:], ACTF.Sin, scale=-2.0 * PI)
                    a = ph32.tile([128, 2 * F], F32, tag="a")
                    nc.scalar.activation(a[:], g[:], ACTF.Abs)
                    C = cs_pool.tile([128, 2 * F], F16, tag="C")
                    nc.scalar.activation(C[:], a[:], ACTF.Sin, bias=hpi[:],
                                         scale=-2.0 * PI)
                    if si == 0:
                        st[ch] = (C, S)
                        continue

                    if (si - 1) in CONST:
                        # preceding merged G already applied the trailing H:
                        # modulate the SBUF f16 state directly
                        reh, imh = st_re, st_im
                    else:
                        pre = psum_pool.tile([128, 2 * F], F32, tag="ps")
                        pim = psum_pool.tile([128, 2 * F], F32, tag="ps")
                        h_mms2(pre, pim, st_re, st_im)
                        reh = hcopy.tile([128, 2 * F], F16, tag="reh")
                        nc.scalar.activation(reh[:], pre[:], ACTF.Copy)
                        imh = hcopy.tile([128, 2 * F], F16, tag="imh")
                        nc.vector.tensor_copy(imh[:], pim[:])

                    p_rc = prod_pool.tile([128, 2 * F], F16, tag="prc")
                    nc.vector.tensor_mul(p_rc[:], reh[:], C[:])
                    p_is = prod_pool.tile([128, 2 * F], F16, tag="pis")
                    nc.vector.tensor_mul(p_is[:], imh[:], S[:])
                    p_rs = fprod_pool.tile([128, 2 * F], F16, tag="prs")
                    nc.vector.tensor_mul(p_rs[:], reh[:], S[:])
                    p_ic = fprod_pool.tile([128, 2 * F], F16, tag="pic")
                    nc.vector.tensor_mul(p_ic[:], imh[:], C[:])
                    st_re = state_pool.tile([128, 2 * F], F16, tag="sre")
                    nc.vector.tensor_sub(st_re[:], p_rc[:], p_is[:])
                    if si in FUSE_ADD:
                        # st_im = p_rs + p_ic folded into the next H-app's
                        # PSUM accumulation (consumer is always an H, not G)
                        st[ch] = (st_re, (p_rs, p_ic))
                    else:
                        # only G-feeding steps reach here; the G matmul order
                        # gives st_im enough slack for the slow Pool engine
                        st_im = state_pool.tile([128, 2 * F], F16, tag="sim")
                        nc.gpsimd.tensor_add(st_im[:], p_rs[:], p_ic[:])
                        st[ch] = (st_re, st_im)

              # final H, probs, Z projection
              for ch in chunk_ids:
                st_re, st_im = st[ch]
                pre = psum_pool.tile([128, 2 * F], F32, tag="ps")
                pim = psum_pool.tile([128, 2 * F], F32, tag="ps")
                h_mms2(pre, pim, st_re, st_im)
                p1 = tail_pool.tile([128, 2 * F], F32, tag="p1")
                nc.scalar.activation(p1[:], pre[:], ACTF.Square, scale=1.0 / 16.0)
                p2 = tail_pool.tile([128, 2 * F], F32, tag="p2")
                nc.scalar.activation(p2[:], pim[:], ACTF.Square, scale=1.0 / 16.0)
                probs = tail_pool.tile([128, 2 * F], F32, tag="probs")
                nc.gpsimd.tensor_add(probs[:], p1[:], p2[:])
                for sub in range(F // 128):
                    zp = psum_pool.tile([128, 8], F32, tag="ps")
                    nc.tensor.matmul(zp[:], probs[:, sub * 128:(sub + 1) * 128],
                                     Z0[:], start=True, stop=False)
                    nc.tensor.matmul(zp[:], probs[:, F + sub * 128: F + (sub + 1) * 128],
                                     Z1[:], start=False, stop=True)
                    zs = tail_pool.tile([128, 8], F32, tag="zs")
                    nc.scalar.activation(zs[:], zp[:], ACTF.Copy)
                    nc.sync.dma_start(
                        out_d[ch * F + sub * 128: ch * F + (sub + 1) * 128, :],
                        zs[:])
    _legalize_waits(nc)
    return nc


_PROGRAM_CACHE = {}


def kernel(observation, theta, omega, _trace=False):
    observation = np.asarray(observation, np.float32)
    W, M, G_blocks, Z = _build_host_tables(theta, omega)
    NX = len(XDEP)
    W_flat = np.ascontiguousarray(
        W.transpose(1, 0, 2).reshape(13, NX * DIM))
    x_augT = np.concatenate(
        [observation, np.ones((B_TOTAL, 1), np.float32)], 1).T  # (13, 16384)
    x_augT = np.ascontiguousarray(x_augT)

    G_flat = np.zeros((128, 4 * 4 * 3 * 128), np.float32)
    for l in range(4):
        for b in range(4):     # hp*2+h
            GrT, GiT = G_blocks[l][b]
            base = ((l * 4 + b) * 3) * 128
            G_flat[:, base:base + 128] = GrT
            G_flat[:, base + 128:base + 256] = GiT
            G_flat[:, base + 256:base + 384] = -GiT

    if "nc" not in _PROGRAM_CACHE:
        _PROGRAM_CACHE["nc"] = _build_program()
    nc = _PROGRAM_CACHE["nc"]

    in_maps = []
    for c in range(N_CORES):
        in_maps.append({
            "xT": np.ascontiguousarray(x_augT[:, c * B_CORE:(c + 1) * B_CORE]),
            "W": W_flat,
            "Mp": M.astype(np.float16),
            "Mn": np.ascontiguousarray(-M).astype(np.float16),
            "G": G_flat.astype(np.float16),
            "Zt": Z,
        })
    res = run_bass_kernel_spmd(nc, in_maps, core_ids=list(range(N_CORES)),
                               trace=_trace)
    out = np.concatenate([r["out"] for r in res.results], 0)
    if _trace:
        kernel.last_results = res
    return out
